# revision 11
# baseline (speedup 1.0000x reference)
"""Trainium2 Bass kernel for a dense transformer encoder block.

Sharding (8 cores): sequence-parallel. Core c handles batch b = c//4 and the
512-token query slice q0 = (c%4)*512. Each core computes K/V projections for
its full batch (duplicated across the 4 cores of a batch), attention for its
own queries over all 16 heads, then the FFN for its token slice. No
collectives; the host pre-transposes x / the mask and gathers the outputs.

v2 schedule: the streaming operands (x, weights, K/Q/V tiles, exp(scores))
are bf16 (full PE rate, half DMA), residual/LN math stays f32. The mask is
multiplicative 0/1 applied after exp on the DVE in 4x bf16 mode. The scores
loop is software-pipelined (AV trails scores by one 2-chunk group) and the
next quarter's K/V/Q projection chains are interleaved into the current
quarter's Act-limited scores phase. Softmax normalization folds into the AV
matmul via an appended ones column on V; the denominator broadcast runs on
the otherwise-idle GpSimd engine.
"""

import sys
from contextlib import ExitStack

import numpy as np

for _p in ("/opt/trn_rl_repo", "/opt/pypackages"):
    if _p not in sys.path:
        sys.path.append(_p)

import ml_dtypes  # noqa: E402
import concourse.bass as bass  # noqa: E402
import concourse.tile as tile  # noqa: E402
from concourse import bacc, mybir  # noqa: E402
from concourse.masks import make_identity  # noqa: E402

F32 = mybir.dt.float32
F32R = mybir.dt.float32r
BF16 = mybir.dt.bfloat16
AF = mybir.ActivationFunctionType
ALU = mybir.AluOpType

P = 128
DH = 64            # head dim (fixed)
DFH = DH + 1       # head dim + ones column
LN_EPS = 1e-5
GK = 2             # score kt-chunks per pipeline group

FULL_CFG = dict(B=2, L=2048, D=1024, H=16, DFF=4096, NCORES=8)


def build_bass(cfg):
    B, L, D, H, DFF = cfg["B"], cfg["L"], cfg["D"], cfg["H"], cfg["DFF"]
    NCORES = cfg["NCORES"]
    CPB = NCORES // B          # cores per batch
    TOK = L // CPB             # tokens per core
    KC = D // P                # contraction chunks over D
    KT = L // P                # key-token 128-chunks
    NG = KT // GK              # score pipeline groups per head
    HPQ = 4                    # heads per quarter
    NQ = H // HPQ              # quarters
    NTQ = TOK // P             # query-token 128-chunks per core
    FFC = DFF // P             # 128-chunks over DFF
    BNF = min(512, D)          # bn_stats subgroup
    NBN = D // BNF
    assert H * DH == D and TOK % P == 0 and L % 512 == 0

    nc = bacc.Bacc(None, target_bir_lowering=False, debug=False)
    with tile.TileContext(nc) as tc, ExitStack() as top, \
            nc.allow_low_precision(reason="bf16/fp32r streaming operands"):
        dram = top.enter_context(tc.tile_pool(name="dram", bufs=1, space="DRAM"))

        def din(name, shape, dtype=F32):
            return dram.tile(shape, dtype, kind="ExternalInput", name=name,
                             uniquify=False)

        xt_d = din("xt", [D, L], BF16)          # x[b].T
        xtq_d = din("xtq", [D, TOK], BF16)      # x[b, q0:q0+TOK].T
        xq_d = din("xq", [TOK, D])              # x[b, q0:q0+TOK]
        m01_d = din("m01", [L, TOK], BF16)      # 0/1 mask, transposed slice
        wq_d = din("wq", [D, D], BF16)
        wk_d = din("wk", [D, D], BF16)
        wv_d = din("wv", [D, D], BF16)
        wo_d = din("wo", [D, D], BF16)
        w1_d = din("w1", [D, DFF], BF16)
        w2_d = din("w2", [DFF, D], BF16)
        bq_d = din("bq", [D])
        bk_d = din("bk", [D])
        bv_d = din("bv", [D])
        bo_d = din("bo", [D])
        b1_d = din("b1", [DFF])
        b2_d = din("b2", [D])
        g1_d = din("g1", [D])
        be1_d = din("be1", [D])
        g2_d = din("g2", [D])
        be2_d = din("be2", [D])
        out_d = dram.tile([TOK, D], F32, kind="ExternalOutput", name="out",
                          uniquify=False)

        def bcast_row(src_ap):
            # DRAM [n] row -> AP broadcasting to P partitions
            return bass.AP(tensor=src_ap.tensor, offset=src_ap.offset,
                           ap=[[0, P]] + [list(a) for a in src_ap.ap])

        const = top.enter_context(tc.tile_pool(name="const", bufs=1))
        ident = const.tile([P, P], F32, name="ident")
        make_identity(nc, ident)
        eps_t = const.tile([P, 1], F32, name="eps_t")
        nc.vector.memset(eps_t[:], LN_EPS)
        bq_sb = const.tile([P, KC], F32, name="bq_sb")
        nc.sync.dma_start(out=bq_sb, in_=bq_d[:].rearrange("(c p) -> p c", p=P))
        bk_sb = const.tile([P, KC], F32, name="bk_sb")
        nc.sync.dma_start(out=bk_sb, in_=bk_d[:].rearrange("(c p) -> p c", p=P))
        b1_sb = const.tile([P, FFC], F32, name="b1_sb")
        nc.sync.dma_start(out=b1_sb, in_=b1_d[:].rearrange("(c p) -> p c", p=P))
        vb_bc = const.tile([P, H, DH], F32, name="vb_bc")
        nc.sync.dma_start(
            out=vb_bc,
            in_=bcast_row(bv_d[:].rearrange("(h d) -> h d", d=DH)))
        onesd = const.tile([P, DH], F32, name="onesd")
        nc.vector.memset(onesd[:], 1.0)

        # mid-lifetime: attention outputs + wo (consumed by the tail)
        mid = top.enter_context(tc.tile_pool(name="mid", bufs=1))
        ot = [mid.tile([DH, TOK], BF16, name=f"ot{h}", tag=f"ot{h}")
              for h in range(H)]
        wo_hm = mid.tile([DH, H, D], BF16, name="wo_hm")

        # ---------------- attention (projections + scores + AV) ------------
        with ExitStack() as attn:
            pa = attn.enter_context(tc.tile_pool(name="attn_sb", bufs=1))
            pw = attn.enter_context(tc.tile_pool(name="attn_wk", bufs=1))
            ps = attn.enter_context(tc.tile_pool(name="attn_ps", bufs=1,
                                                 space="PSUM"))

            # shared inputs; xt split into 512-column blocks so the first
            # projection chains unblock before the whole transfer lands
            wq_r = wq_d[:].rearrange("(c p) n -> p c n", p=P)
            wk_r = wk_d[:].rearrange("(c p) n -> p c n", p=P)
            wv_r = wv_d[:].rearrange("(c p) n -> p c n", p=P)

            def load_qweights(hq):
                cs, ce = hq * HPQ * DH, (hq + 1) * HPQ * DH
                wkq = pa.tile([P, KC, HPQ * DH], BF16, name="wkq", tag="wkq",
                              bufs=2)
                nc.sync.dma_start(out=wkq, in_=wk_r[:, :, cs:ce])
                wvq = pa.tile([P, KC, HPQ * DH], BF16, name="wvq", tag="wvq",
                              bufs=2)
                nc.sync.dma_start(out=wvq, in_=wv_r[:, :, cs:ce])
                wqt = pa.tile([P, KC, HPQ * DH], BF16, name="wqt", tag="wqt",
                              bufs=2)
                nc.sync.dma_start(out=wqt, in_=wq_r[:, :, cs:ce])
                return wkq, wvq, wqt

            w0 = load_qweights(0)
            xt = pa.tile([P, KC, L], BF16, name="xt", tag="xt")
            xt_r = xt_d[:].rearrange("(c p) l -> p c l", p=P)
            for blk in range(L // 512):
                sl = slice(blk * 512, (blk + 1) * 512)
                nc.sync.dma_start(out=xt[:, :, sl], in_=xt_r[:, :, sl])
            xtq = pa.tile([P, KC, TOK], BF16, name="xtq", tag="xtq")
            nc.sync.dma_start(out=xtq,
                              in_=xtq_d[:].rearrange("(c p) l -> p c l", p=P))
            m01 = pa.tile([P, KT, TOK], BF16, name="m01", tag="m01")
            nc.sync.dma_start(out=m01,
                              in_=m01_d[:].rearrange("(t p) q -> p t q", p=P))

            state = {}

            def emit_proj(hq):
                """Projection chains for quarter hq; yields per chain."""
                h0 = hq * HPQ
                wkq, wvq, wqt = w0 if hq == 0 else load_qweights(hq)
                ktq = [pa.tile([P, L], BF16, name=f"ktq{i}", tag=f"ktq{i}",
                               bufs=2) for i in range(2)]
                vaug = pa.tile([P, KT, HPQ, DFH], BF16, name="vaug",
                               tag="vaug", bufs=2)
                qs = [pa.tile([P, TOK], BF16, name=f"qs{i}", tag=f"qs{i}",
                              bufs=2) for i in range(2)]
                state[hq] = dict(ktq=ktq, vaug=vaug, qs=qs)
                nc.vector.memset(vaug[:, :, :, DH:DFH], 1.0)
                # K projection: feature-major, scaled 1/8, +bias
                for i in range(2):
                    pc = (h0 + 2 * i) // 2
                    for tg in range(L // 512):
                        kp = ps.tile([P, TOK], F32, name="pp", tag="pp",
                                     bufs=2)
                        sl = slice(tg * 512, (tg + 1) * 512)
                        for kc in range(KC):
                            nc.tensor.matmul(
                                kp, wkq[:, kc, 2 * i * DH:(2 * i + 2) * DH],
                                xt[:, kc, sl],
                                start=(kc == 0), stop=(kc == KC - 1))
                        nc.vector.tensor_scalar(
                            ktq[i][:, sl], kp, bk_sb[:, pc:pc + 1], 0.125,
                            ALU.add, ALU.mult)
                        yield
                # V projection: token-major into [V|1] layout, +bias
                for tci in range(KT):
                    vp = ps.tile([P, TOK], F32, name="pp", tag="pp", bufs=2)
                    for kc in range(KC):
                        nc.tensor.matmul(
                            vp[:, 0:HPQ * DH],
                            xt[:, kc, tci * P:(tci + 1) * P], wvq[:, kc, :],
                            start=(kc == 0), stop=(kc == KC - 1))
                    nc.vector.tensor_tensor(
                        vaug[:, tci, :, 0:DH],
                        vp[:, 0:HPQ * DH].rearrange("p (h d) -> p h d", d=DH),
                        vb_bc[:, h0:h0 + HPQ, :], ALU.add)
                    yield
                # Q projection
                for i in range(2):
                    pc = (h0 + 2 * i) // 2
                    qp = ps.tile([P, TOK], F32, name="pp", tag="pp", bufs=2)
                    for kc in range(KC):
                        nc.tensor.matmul(
                            qp, wqt[:, kc, 2 * i * DH:(2 * i + 2) * DH],
                            xtq[:, kc, :],
                            start=(kc == 0), stop=(kc == KC - 1))
                    nc.vector.tensor_scalar(
                        qs[i], qp, bq_sb[:, pc:pc + 1], None, ALU.add)
                    yield

            def emit_scores(hq):
                """Pipelined scores/exp/mask/AV + normalize for quarter hq."""
                st = state.pop(hq)
                ktq, vaug, qs = st["ktq"], st["vaug"], st["qs"]
                for hl in range(HPQ):
                    h = hq * HPQ + hl
                    i, s = hl // 2, hl % 2
                    base = slice(s * DH, (s + 1) * DH)
                    otp = ps.tile([P, TOK], F32, name="otp", tag="otp",
                                  bufs=2)
                    es_ring = []
                    for g in range(NG + 1):
                        if g < NG:
                            sp = ps.tile([P, GK, TOK], F32, name="sp",
                                         tag="sp", bufs=2)
                            for j in range(GK):
                                kt = GK * g + j
                                nc.tensor.matmul(
                                    sp[:, j, :],
                                    ktq[i][base, kt * P:(kt + 1) * P],
                                    qs[i][base, :], start=True, stop=True)
                            es = pw.tile([P, GK, TOK], BF16, name="es",
                                         tag="es", bufs=3)
                            nc.scalar.activation(es, sp, AF.Exp)
                            nc.vector.tensor_tensor(
                                es, es, m01[:, GK * g:GK * (g + 1), :],
                                ALU.mult)
                            es_ring.append(es)
                        if g > 0:
                            esp = es_ring[g - 1]
                            for j in range(GK):
                                kt = GK * (g - 1) + j
                                nc.tensor.matmul(
                                    otp[0:DFH, :], vaug[:, kt, hl, :],
                                    esp[:, j, :],
                                    start=(kt == 0), stop=(kt == KT - 1))
                        yield
                    # normalize: recip of denominator row, broadcast across
                    # partitions via a ones-column matmul, multiply into the
                    # head's output tile (tensor_tensor can't take two PSUM
                    # inputs, so the broadcast bounces through SBUF)
                    rt = pw.tile([P, TOK], F32, name="rt", tag="rt", bufs=2)
                    nc.vector.reciprocal(rt[DH:DFH, :], otp[DH:DFH, :])
                    rb = ps.tile([P, TOK], F32, name="rb", tag="otp", bufs=2)
                    nc.tensor.matmul(rb[0:DH, :], onesd[DH:DFH, :],
                                     rt[DH:DFH, :], start=True, stop=True)
                    rbs = pw.tile([DH, TOK], F32, name="rbs", tag="rbs",
                                  bufs=2)
                    nc.scalar.activation(rbs, rb[0:DH, :], AF.Copy)
                    nc.vector.tensor_tensor(ot[h][:], otp[0:DH, :],
                                            rbs, ALU.mult)
                    yield

            def interleave(a, b):
                alive = [a, b]
                while alive:
                    for g in list(alive):
                        try:
                            next(g)
                        except StopIteration:
                            alive.remove(g)

            for _ in emit_proj(0):
                pass
            for hq in range(1, NQ):
                interleave(emit_scores(hq - 1), emit_proj(hq))
            # prefetch tail weights while the last quarter's scores drain
            nc.sync.dma_start(
                out=wo_hm,
                in_=wo_d[:].rearrange("(h p) n -> p h n", p=DH))
            for _ in emit_scores(NQ - 1):
                pass

        # ---------------- O-projection + LN1 + transpose + FFN --------------
        with ExitStack() as tail:
            pcd = tail.enter_context(tc.tile_pool(name="cd_sb", bufs=1))
            pwk = tail.enter_context(tc.tile_pool(name="cd_wk", bufs=1))
            h_t = [pcd.tile([P, D], F32, name=f"h{t}", tag=f"h{t}")
                   for t in range(NTQ)]
            hT = [pcd.tile([P, TOK], BF16, name=f"hT{c}", tag=f"hT{c}")
                  for c in range(KC)]
            f1 = pcd.tile([P, FFC, TOK], BF16, name="f1", tag="f1")
            xq = pcd.tile([P, NTQ, D], F32, name="xq", tag="xq")
            nc.sync.dma_start(out=xq,
                              in_=xq_d[:].rearrange("(t p) d -> p t d", p=P))
            bo_bc = pcd.tile([P, D], F32, name="bo_bc")
            nc.sync.dma_start(out=bo_bc, in_=bcast_row(bo_d[:]))
            b2_bc = pcd.tile([P, D], F32, name="b2_bc")
            nc.sync.dma_start(out=b2_bc, in_=bcast_row(b2_d[:]))
            g1_bc = pcd.tile([P, D], F32, name="g1_bc")
            nc.sync.dma_start(out=g1_bc, in_=bcast_row(g1_d[:]))
            be1_bc = pcd.tile([P, D], F32, name="be1_bc")
            nc.sync.dma_start(out=be1_bc, in_=bcast_row(be1_d[:]))
            g2_bc = pcd.tile([P, D], F32, name="g2_bc")
            nc.sync.dma_start(out=g2_bc, in_=bcast_row(g2_d[:]))
            be2_bc = pcd.tile([P, D], F32, name="be2_bc")
            nc.sync.dma_start(out=be2_bc, in_=bcast_row(be2_d[:]))

            def layernorm(x_tile, g_bc, b_bc, wk):
                st = wk.tile([P, NBN, 6], F32, name="lnst", tag="lnst",
                             bufs=2)
                xv = x_tile.rearrange("p (s f) -> p s f", f=BNF)
                for sg in range(NBN):
                    nc.vector.bn_stats(out=st[:, sg, :], in_=xv[:, sg, :])
                mv = wk.tile([P, 2], F32, name="lnmv", tag="lnmv", bufs=2)
                nc.vector.bn_aggr(out=mv, in_=st)
                sq = wk.tile([P, 1], F32, name="lnsq", tag="lnsq", bufs=2)
                nc.scalar.activation(sq, mv[:, 1:2], AF.Sqrt, bias=eps_t)
                nc.vector.reciprocal(sq, sq)
                nc.vector.tensor_scalar(x_tile, x_tile, mv[:, 0:1], sq,
                                        ALU.subtract, ALU.mult)
                nc.vector.tensor_tensor(x_tile, x_tile, g_bc, ALU.mult)
                nc.vector.tensor_tensor(x_tile, x_tile, b_bc, ALU.add)

            # O-projection (per-head contraction 64) + residual + LN1 +
            # transpose to feature-major, pipelined per token tile
            with tc.tile_pool(name="cd_ps1", bufs=1, space="PSUM") as pps:
                for t in range(NTQ):
                    for dc in range(2):
                        op = pps.tile([P, 512], F32, name="op", tag="op",
                                      bufs=4)
                        dsl = slice(dc * 512, (dc + 1) * 512)
                        for h in range(H):
                            nc.tensor.matmul(op, ot[h][:, t * P:(t + 1) * P],
                                             wo_hm[:, h, dsl],
                                             start=(h == 0), stop=(h == H - 1))
                        nc.vector.tensor_tensor(h_t[t][:, dsl], op,
                                                xq[:, t, dsl], ALU.add)
                        nc.vector.tensor_tensor(h_t[t][:, dsl],
                                                h_t[t][:, dsl],
                                                bo_bc[:, dsl], ALU.add)
                    layernorm(h_t[t], g1_bc, be1_bc, pwk)
                    for c in range(KC):
                        tp = pps.tile([P, P], F32, name="tp", tag="tp",
                                      bufs=2)
                        nc.tensor.transpose(tp, h_t[t][:, c * P:(c + 1) * P],
                                            ident)
                        nc.scalar.activation(hT[c][:, t * P:(t + 1) * P], tp,
                                             AF.Copy)

            # FFN mm1 + ReLU (feature-major f1)
            pps = tail.enter_context(tc.tile_pool(name="cd_ps2", bufs=1,
                                                  space="PSUM"))
            w1_r = w1_d[:].rearrange("(c p) f -> p c f", p=P)
            for fq in range(DFF // 512):
                w1t = pwk.tile([P, KC, 512], BF16, name="w1t", tag="w1t",
                               bufs=2)
                nc.sync.dma_start(out=w1t,
                                  in_=w1_r[:, :, fq * 512:(fq + 1) * 512])
                for ffl in range(4):
                    ff = fq * 4 + ffl
                    fp = pps.tile([P, TOK], F32, name="fp", tag="fp", bufs=2)
                    for kc in range(KC):
                        nc.tensor.matmul(fp, w1t[:, kc, ffl * P:(ffl + 1) * P],
                                         hT[kc], start=(kc == 0),
                                         stop=(kc == KC - 1))
                    nc.scalar.activation(f1[:, ff, :], fp, AF.Relu,
                                         bias=b1_sb[:, ff:ff + 1])

            # FFN mm2 + residual + LN2 + store
            w2_r = w2_d[:].rearrange("(c p) n -> p c n", p=P)
            f2 = [pcd.tile([P, D], F32, name=f"f2_{t}", tag=f"f2_{t}")
                  for t in range(NTQ)]
            for dc in range(2):
                dsl = slice(dc * 512, (dc + 1) * 512)
                g2p = [pps.tile([P, 512], F32, name=f"g2p{t}", tag=f"g2p{t}",
                                bufs=1) for t in range(NTQ)]
                for kg in range(FFC // 4):
                    w2t = pwk.tile([P, 4, 512], BF16, name="w2t", tag="w2t",
                                   bufs=3)
                    nc.sync.dma_start(out=w2t,
                                      in_=w2_r[:, 4 * kg:4 * (kg + 1), dsl])
                    for kl in range(4):
                        kc2 = 4 * kg + kl
                        for t in range(NTQ):
                            nc.tensor.matmul(
                                g2p[t], f1[:, kc2, t * P:(t + 1) * P],
                                w2t[:, kl, :], start=(kc2 == 0),
                                stop=(kc2 == FFC - 1))
                for t in range(NTQ):
                    nc.vector.tensor_tensor(f2[t][:, dsl], g2p[t],
                                            h_t[t][:, dsl], ALU.add)
                    nc.vector.tensor_tensor(f2[t][:, dsl], f2[t][:, dsl],
                                            b2_bc[:, dsl], ALU.add)
            for t in range(NTQ):
                layernorm(f2[t], g2_bc, be2_bc, pwk)
                nc.sync.dma_start(out=out_d[t * P:(t + 1) * P, :], in_=f2[t])

    nc.compile()
    return nc


def make_in_maps(cfg, inp):
    """Build per-core input dicts from full (host) inputs."""
    B, L, D, H = cfg["B"], cfg["L"], cfg["D"], cfg["H"]
    NCORES = cfg["NCORES"]
    CPB = NCORES // B
    TOK = L // CPB
    f32 = np.float32
    bf16 = ml_dtypes.bfloat16
    x = np.asarray(inp["x"], f32)
    mask = np.asarray(inp["mask"], bool)
    w = {k: np.asarray(inp[k], f32) for k in
         ("wq", "bq", "wk", "bk", "wv", "bv", "wo", "bo", "w1", "b1",
          "w2", "b2", "ln1_g", "ln1_b", "ln2_g", "ln2_b")}
    shared = dict(wq=w["wq"].astype(bf16), wk=w["wk"].astype(bf16),
                  wv=w["wv"].astype(bf16), wo=w["wo"].astype(bf16),
                  w1=w["w1"].astype(bf16), w2=w["w2"].astype(bf16),
                  bq=w["bq"], bk=w["bk"], bv=w["bv"], bo=w["bo"],
                  b1=w["b1"], b2=w["b2"],
                  g1=w["ln1_g"], be1=w["ln1_b"], g2=w["ln2_g"],
                  be2=w["ln2_b"])
    shared = {k: np.ascontiguousarray(v) for k, v in shared.items()}
    in_maps = []
    for c in range(NCORES):
        b, q0 = c // CPB, (c % CPB) * TOK
        xb = x[b]
        m01 = np.where(mask[b, q0:q0 + TOK, :].T, bf16(0.0), bf16(1.0))
        m = dict(shared)
        m["xt"] = np.ascontiguousarray(xb.T.astype(bf16))
        m["xtq"] = np.ascontiguousarray(xb[q0:q0 + TOK].T.astype(bf16))
        m["xq"] = np.ascontiguousarray(xb[q0:q0 + TOK])
        m["m01"] = np.ascontiguousarray(m01.astype(bf16))
        in_maps.append(m)
    return in_maps


_NC_CACHE = {}
TRACE = False
LAST_RESULTS = None


def _get_nc(key, cfg):
    if key not in _NC_CACHE:
        _NC_CACHE[key] = build_bass(cfg)
    return _NC_CACHE[key]


def kernel(**inputs):
    global LAST_RESULTS
    from concourse.bass_utils import run_bass_kernel_spmd

    cfg = FULL_CFG
    B, L, D = cfg["B"], cfg["L"], cfg["D"]
    NCORES = cfg["NCORES"]
    CPB = NCORES // B
    TOK = L // CPB
    nc = _get_nc("full", cfg)
    in_maps = make_in_maps(cfg, inputs)
    res = run_bass_kernel_spmd(nc, in_maps, core_ids=list(range(NCORES)),
                               trace=TRACE)
    LAST_RESULTS = res
    out = np.empty((B, L, D), np.float32)
    for c in range(NCORES):
        b, q0 = c // CPB, (c % CPB) * TOK
        out[b, q0:q0 + TOK] = res.results[c]["out"]
    return out


# revision 12
# speedup vs baseline: 1.0876x; 1.0876x over previous
"""Trainium2 Bass kernel for a dense transformer encoder block.

Sharding (8 cores): sequence-parallel. Core c handles batch b = c//4 and the
512-token query slice q0 = (c%4)*512. Each core computes K/V projections for
its full batch (duplicated across the 4 cores of a batch), attention for its
own queries over all 16 heads, then the FFN for its token slice. No
collectives; the host pre-transposes x / the mask and gathers the outputs.

v2 schedule: the streaming operands (x, weights, K/Q/V tiles, exp(scores))
are bf16 (full PE rate, half DMA), residual/LN math stays f32. The mask is
multiplicative 0/1 applied after exp on the DVE in 4x bf16 mode. The scores
loop is software-pipelined (AV trails scores by one 2-chunk group) and the
next quarter's K/V/Q projection chains are interleaved into the current
quarter's Act-limited scores phase. Softmax normalization folds into the AV
matmul via an appended ones column on V; the reciprocal denominator row is
broadcast across partitions with a ones-column matmul (f32 — the hardware
verifier rejects mixed 32/16-bit matmul inputs) and bounced through SBUF.
"""

import sys
from contextlib import ExitStack

import numpy as np

for _p in ("/opt/trn_rl_repo", "/opt/pypackages"):
    if _p not in sys.path:
        sys.path.append(_p)

import ml_dtypes  # noqa: E402
import concourse.bass as bass  # noqa: E402
import concourse.tile as tile  # noqa: E402
from concourse import bacc, mybir  # noqa: E402
from concourse.masks import make_identity  # noqa: E402

F32 = mybir.dt.float32
F32R = mybir.dt.float32r
BF16 = mybir.dt.bfloat16
AF = mybir.ActivationFunctionType
ALU = mybir.AluOpType

P = 128
DH = 64            # head dim (fixed)
DFH = DH + 1       # head dim + ones column
LN_EPS = 1e-5
GK = 2             # score kt-chunks per pipeline group

FULL_CFG = dict(B=2, L=2048, D=1024, H=16, DFF=4096, NCORES=8)


def build_bass(cfg):
    B, L, D, H, DFF = cfg["B"], cfg["L"], cfg["D"], cfg["H"], cfg["DFF"]
    NCORES = cfg["NCORES"]
    CPB = NCORES // B          # cores per batch
    TOK = L // CPB             # tokens per core
    KC = D // P                # contraction chunks over D
    KT = L // P                # key-token 128-chunks
    NG = KT // GK              # score pipeline groups per head
    HPQ = 4                    # heads per quarter
    NQ = H // HPQ              # quarters
    NTQ = TOK // P             # query-token 128-chunks per core
    FFC = DFF // P             # 128-chunks over DFF
    BNF = min(512, D)          # bn_stats subgroup
    NBN = D // BNF
    assert H * DH == D and TOK % P == 0 and L % 512 == 0

    nc = bacc.Bacc(None, target_bir_lowering=False, debug=False)
    with tile.TileContext(nc) as tc, ExitStack() as top, \
            nc.allow_low_precision(reason="bf16/fp32r streaming operands"):
        dram = top.enter_context(tc.tile_pool(name="dram", bufs=1, space="DRAM"))

        def din(name, shape, dtype=F32):
            return dram.tile(shape, dtype, kind="ExternalInput", name=name,
                             uniquify=False)

        xt_d = din("xt", [D, L], BF16)          # x[b].T
        xtq_d = din("xtq", [D, TOK], BF16)      # x[b, q0:q0+TOK].T
        xq_d = din("xq", [TOK, D])              # x[b, q0:q0+TOK]
        m01_d = din("m01", [L, TOK], BF16)      # 0/1 mask, transposed slice
        wq_d = din("wq", [D, D], BF16)
        wk_d = din("wk", [D, D], BF16)
        wv_d = din("wv", [D, D], BF16)
        wo_d = din("wo", [D, D], BF16)
        w1_d = din("w1", [D, DFF], BF16)
        w2_d = din("w2", [DFF, D], BF16)
        bq_d = din("bq", [D])
        bk_d = din("bk", [D])
        bv_d = din("bv", [D])
        bo_d = din("bo", [D])
        b1_d = din("b1", [DFF])
        b2_d = din("b2", [D])
        g1_d = din("g1", [D])
        be1_d = din("be1", [D])
        g2_d = din("g2", [D])
        be2_d = din("be2", [D])
        out_d = dram.tile([TOK, D], F32, kind="ExternalOutput", name="out",
                          uniquify=False)

        def bcast_row(src_ap):
            # DRAM [n] row -> AP broadcasting to P partitions
            return bass.AP(tensor=src_ap.tensor, offset=src_ap.offset,
                           ap=[[0, P]] + [list(a) for a in src_ap.ap])

        const = top.enter_context(tc.tile_pool(name="const", bufs=1))
        ident = const.tile([P, P], F32, name="ident")
        make_identity(nc, ident)
        eps_t = const.tile([P, 1], F32, name="eps_t")
        nc.vector.memset(eps_t[:], LN_EPS)
        bq_sb = const.tile([P, KC], F32, name="bq_sb")
        nc.sync.dma_start(out=bq_sb, in_=bq_d[:].rearrange("(c p) -> p c", p=P))
        bk_sb = const.tile([P, KC], F32, name="bk_sb")
        nc.sync.dma_start(out=bk_sb, in_=bk_d[:].rearrange("(c p) -> p c", p=P))
        b1_sb = const.tile([P, FFC], F32, name="b1_sb")
        nc.sync.dma_start(out=b1_sb, in_=b1_d[:].rearrange("(c p) -> p c", p=P))
        vb_bc = const.tile([P, H, DH], F32, name="vb_bc")
        nc.sync.dma_start(
            out=vb_bc,
            in_=bcast_row(bv_d[:].rearrange("(h d) -> h d", d=DH)))
        onesd = const.tile([P, DH], F32, name="onesd")
        nc.vector.memset(onesd[:], 1.0)

        # mid-lifetime: attention outputs + wo (consumed by the tail)
        mid = top.enter_context(tc.tile_pool(name="mid", bufs=1))
        ot = [mid.tile([DH, TOK], BF16, name=f"ot{h}", tag=f"ot{h}")
              for h in range(H)]
        wo_hm = mid.tile([DH, H, D], BF16, name="wo_hm")

        # ---------------- attention (projections + scores + AV) ------------
        with ExitStack() as attn:
            pa = attn.enter_context(tc.tile_pool(name="attn_sb", bufs=1))
            pw = attn.enter_context(tc.tile_pool(name="attn_wk", bufs=1))
            ps = attn.enter_context(tc.tile_pool(name="attn_ps", bufs=1,
                                                 space="PSUM"))

            # shared inputs; xt split into 512-column blocks so the first
            # projection chains unblock before the whole transfer lands
            wq_r = wq_d[:].rearrange("(c p) n -> p c n", p=P)
            wk_r = wk_d[:].rearrange("(c p) n -> p c n", p=P)
            wv_r = wv_d[:].rearrange("(c p) n -> p c n", p=P)

            def load_qweights(hq):
                cs, ce = hq * HPQ * DH, (hq + 1) * HPQ * DH
                wkq = pa.tile([P, KC, HPQ * DH], BF16, name="wkq", tag="wkq",
                              bufs=2)
                nc.sync.dma_start(out=wkq, in_=wk_r[:, :, cs:ce])
                wvq = pa.tile([P, KC, HPQ * DH], BF16, name="wvq", tag="wvq",
                              bufs=2)
                nc.sync.dma_start(out=wvq, in_=wv_r[:, :, cs:ce])
                wqt = pa.tile([P, KC, HPQ * DH], BF16, name="wqt", tag="wqt",
                              bufs=2)
                nc.sync.dma_start(out=wqt, in_=wq_r[:, :, cs:ce])
                return wkq, wvq, wqt

            w0 = load_qweights(0)
            xt = pa.tile([P, KC, L], BF16, name="xt", tag="xt")
            xt_r = xt_d[:].rearrange("(c p) l -> p c l", p=P)
            for blk in range(L // 512):
                sl = slice(blk * 512, (blk + 1) * 512)
                nc.sync.dma_start(out=xt[:, :, sl], in_=xt_r[:, :, sl])
            xtq = pa.tile([P, KC, TOK], BF16, name="xtq", tag="xtq")
            nc.sync.dma_start(out=xtq,
                              in_=xtq_d[:].rearrange("(c p) l -> p c l", p=P))
            m01 = pa.tile([P, KT, TOK], BF16, name="m01", tag="m01")
            nc.sync.dma_start(out=m01,
                              in_=m01_d[:].rearrange("(t p) q -> p t q", p=P))

            state = {}

            def emit_proj(hq):
                """Projection chains for quarter hq; yields per chain."""
                h0 = hq * HPQ
                wkq, wvq, wqt = w0 if hq == 0 else load_qweights(hq)
                ktq = [pa.tile([P, L], BF16, name=f"ktq{i}", tag=f"ktq{i}",
                               bufs=2) for i in range(2)]
                vaug = pa.tile([P, KT, HPQ, DFH], BF16, name="vaug",
                               tag="vaug", bufs=2)
                qs = [pa.tile([P, TOK], BF16, name=f"qs{i}", tag=f"qs{i}",
                              bufs=2) for i in range(2)]
                state[hq] = dict(ktq=ktq, vaug=vaug, qs=qs)
                nc.vector.memset(vaug[:, :, :, DH:DFH], 1.0)
                # K projection: feature-major, scaled 1/8, +bias
                for i in range(2):
                    pc = (h0 + 2 * i) // 2
                    for tg in range(L // 512):
                        kp = ps.tile([P, TOK], F32, name="pp", tag="pp",
                                     bufs=2)
                        sl = slice(tg * 512, (tg + 1) * 512)
                        for kc in range(KC):
                            nc.tensor.matmul(
                                kp, wkq[:, kc, 2 * i * DH:(2 * i + 2) * DH],
                                xt[:, kc, sl],
                                start=(kc == 0), stop=(kc == KC - 1))
                        nc.vector.tensor_scalar(
                            ktq[i][:, sl], kp, bk_sb[:, pc:pc + 1], 0.125,
                            ALU.add, ALU.mult)
                        yield
                # V projection: token-major into [V|1] layout, +bias
                for tci in range(KT):
                    vp = ps.tile([P, TOK], F32, name="pp", tag="pp", bufs=2)
                    for kc in range(KC):
                        nc.tensor.matmul(
                            vp[:, 0:HPQ * DH],
                            xt[:, kc, tci * P:(tci + 1) * P], wvq[:, kc, :],
                            start=(kc == 0), stop=(kc == KC - 1))
                    nc.vector.tensor_tensor(
                        vaug[:, tci, :, 0:DH],
                        vp[:, 0:HPQ * DH].rearrange("p (h d) -> p h d", d=DH),
                        vb_bc[:, h0:h0 + HPQ, :], ALU.add)
                    yield
                # Q projection
                for i in range(2):
                    pc = (h0 + 2 * i) // 2
                    qp = ps.tile([P, TOK], F32, name="pp", tag="pp", bufs=2)
                    for kc in range(KC):
                        nc.tensor.matmul(
                            qp, wqt[:, kc, 2 * i * DH:(2 * i + 2) * DH],
                            xtq[:, kc, :],
                            start=(kc == 0), stop=(kc == KC - 1))
                    nc.vector.tensor_scalar(
                        qs[i], qp, bq_sb[:, pc:pc + 1], None, ALU.add)
                    yield

            def emit_scores(hq):
                """Pipelined scores/exp/mask/AV + normalize for quarter hq."""
                st = state.pop(hq)
                ktq, vaug, qs = st["ktq"], st["vaug"], st["qs"]
                for hl in range(HPQ):
                    h = hq * HPQ + hl
                    i, s = hl // 2, hl % 2
                    base = slice(s * DH, (s + 1) * DH)
                    otp = ps.tile([P, TOK], F32, name="otp", tag="otp",
                                  bufs=2)
                    es_ring = []
                    for g in range(NG + 1):
                        if g < NG:
                            sp = ps.tile([P, GK, TOK], F32, name="sp",
                                         tag="sp", bufs=2)
                            for j in range(GK):
                                kt = GK * g + j
                                nc.tensor.matmul(
                                    sp[:, j, :],
                                    ktq[i][base, kt * P:(kt + 1) * P],
                                    qs[i][base, :], start=True, stop=True)
                            es = pw.tile([P, GK, TOK], BF16, name="es",
                                         tag="es", bufs=3)
                            nc.scalar.activation(es, sp, AF.Exp)
                            nc.vector.tensor_tensor(
                                es, es, m01[:, GK * g:GK * (g + 1), :],
                                ALU.mult)
                            es_ring.append(es)
                        if g > 0:
                            esp = es_ring[g - 1]
                            for j in range(GK):
                                kt = GK * (g - 1) + j
                                nc.tensor.matmul(
                                    otp[0:DFH, :], vaug[:, kt, hl, :],
                                    esp[:, j, :],
                                    start=(kt == 0), stop=(kt == KT - 1))
                        yield
                    # normalize: recip of denominator row, broadcast across
                    # partitions via a ones-column matmul, multiply into the
                    # head's output tile (tensor_tensor can't take two PSUM
                    # inputs, so the broadcast bounces through SBUF)
                    rt = pw.tile([P, TOK], F32, name="rt", tag="rt", bufs=2)
                    nc.vector.reciprocal(rt[DH:DFH, :], otp[DH:DFH, :])
                    rb = ps.tile([P, TOK], F32, name="rb", tag="otp", bufs=2)
                    nc.tensor.matmul(rb[0:DH, :], onesd[DH:DFH, :],
                                     rt[DH:DFH, :], start=True, stop=True)
                    rbs = pw.tile([DH, TOK], F32, name="rbs", tag="rbs",
                                  bufs=2)
                    nc.scalar.activation(rbs, rb[0:DH, :], AF.Copy)
                    nc.vector.tensor_tensor(ot[h][:], otp[0:DH, :],
                                            rbs, ALU.mult)
                    yield

            def interleave(a, b):
                alive = [a, b]
                while alive:
                    for g in list(alive):
                        try:
                            next(g)
                        except StopIteration:
                            alive.remove(g)

            for _ in emit_proj(0):
                pass
            for hq in range(1, NQ):
                interleave(emit_scores(hq - 1), emit_proj(hq))
            # prefetch tail weights while the last quarter's scores drain
            nc.sync.dma_start(
                out=wo_hm,
                in_=wo_d[:].rearrange("(h p) n -> p h n", p=DH))
            for _ in emit_scores(NQ - 1):
                pass

        # ---------------- O-projection + LN1 + transpose + FFN --------------
        with ExitStack() as tail:
            pcd = tail.enter_context(tc.tile_pool(name="cd_sb", bufs=1))
            pwk = tail.enter_context(tc.tile_pool(name="cd_wk", bufs=1))
            h_t = [pcd.tile([P, D], F32, name=f"h{t}", tag=f"h{t}")
                   for t in range(NTQ)]
            hT = [pcd.tile([P, TOK], BF16, name=f"hT{c}", tag=f"hT{c}")
                  for c in range(KC)]
            f1 = pcd.tile([P, FFC, TOK], BF16, name="f1", tag="f1")
            xq = pcd.tile([P, NTQ, D], F32, name="xq", tag="xq")
            nc.sync.dma_start(out=xq,
                              in_=xq_d[:].rearrange("(t p) d -> p t d", p=P))
            bo_bc = pcd.tile([P, D], F32, name="bo_bc")
            nc.sync.dma_start(out=bo_bc, in_=bcast_row(bo_d[:]))
            b2_bc = pcd.tile([P, D], F32, name="b2_bc")
            nc.sync.dma_start(out=b2_bc, in_=bcast_row(b2_d[:]))
            g1_bc = pcd.tile([P, D], F32, name="g1_bc")
            nc.sync.dma_start(out=g1_bc, in_=bcast_row(g1_d[:]))
            be1_bc = pcd.tile([P, D], F32, name="be1_bc")
            nc.sync.dma_start(out=be1_bc, in_=bcast_row(be1_d[:]))
            g2_bc = pcd.tile([P, D], F32, name="g2_bc")
            nc.sync.dma_start(out=g2_bc, in_=bcast_row(g2_d[:]))
            be2_bc = pcd.tile([P, D], F32, name="be2_bc")
            nc.sync.dma_start(out=be2_bc, in_=bcast_row(be2_d[:]))

            def layernorm(x_tile, g_bc, b_bc, wk):
                st = wk.tile([P, NBN, 6], F32, name="lnst", tag="lnst",
                             bufs=2)
                xv = x_tile.rearrange("p (s f) -> p s f", f=BNF)
                for sg in range(NBN):
                    nc.vector.bn_stats(out=st[:, sg, :], in_=xv[:, sg, :])
                mv = wk.tile([P, 2], F32, name="lnmv", tag="lnmv", bufs=2)
                nc.vector.bn_aggr(out=mv, in_=st)
                sq = wk.tile([P, 1], F32, name="lnsq", tag="lnsq", bufs=2)
                nc.scalar.activation(sq, mv[:, 1:2], AF.Sqrt, bias=eps_t)
                nc.vector.reciprocal(sq, sq)
                nc.vector.tensor_scalar(x_tile, x_tile, mv[:, 0:1], sq,
                                        ALU.subtract, ALU.mult)
                nc.vector.tensor_tensor(x_tile, x_tile, g_bc, ALU.mult)
                nc.vector.tensor_tensor(x_tile, x_tile, b_bc, ALU.add)

            # O-projection (per-head contraction 64) + residual + LN1 +
            # transpose to feature-major, pipelined per token tile
            with tc.tile_pool(name="cd_ps1", bufs=1, space="PSUM") as pps:
                for t in range(NTQ):
                    for dc in range(2):
                        op = pps.tile([P, 512], F32, name="op", tag="op",
                                      bufs=4)
                        dsl = slice(dc * 512, (dc + 1) * 512)
                        for h in range(H):
                            nc.tensor.matmul(op, ot[h][:, t * P:(t + 1) * P],
                                             wo_hm[:, h, dsl],
                                             start=(h == 0), stop=(h == H - 1))
                        nc.vector.tensor_tensor(h_t[t][:, dsl], op,
                                                xq[:, t, dsl], ALU.add)
                        nc.vector.tensor_tensor(h_t[t][:, dsl],
                                                h_t[t][:, dsl],
                                                bo_bc[:, dsl], ALU.add)
                    layernorm(h_t[t], g1_bc, be1_bc, pwk)
                    for c in range(KC):
                        tp = pps.tile([P, P], F32, name="tp", tag="tp",
                                      bufs=2)
                        nc.tensor.transpose(tp, h_t[t][:, c * P:(c + 1) * P],
                                            ident)
                        nc.scalar.activation(hT[c][:, t * P:(t + 1) * P], tp,
                                             AF.Copy)

            # FFN mm1 + ReLU (feature-major f1)
            pps = tail.enter_context(tc.tile_pool(name="cd_ps2", bufs=1,
                                                  space="PSUM"))
            w1_r = w1_d[:].rearrange("(c p) f -> p c f", p=P)
            for fq in range(DFF // 512):
                w1t = pwk.tile([P, KC, 512], BF16, name="w1t", tag="w1t",
                               bufs=2)
                nc.sync.dma_start(out=w1t,
                                  in_=w1_r[:, :, fq * 512:(fq + 1) * 512])
                for ffl in range(4):
                    ff = fq * 4 + ffl
                    fp = pps.tile([P, TOK], F32, name="fp", tag="fp", bufs=2)
                    for kc in range(KC):
                        nc.tensor.matmul(fp, w1t[:, kc, ffl * P:(ffl + 1) * P],
                                         hT[kc], start=(kc == 0),
                                         stop=(kc == KC - 1))
                    nc.scalar.activation(f1[:, ff, :], fp, AF.Relu,
                                         bias=b1_sb[:, ff:ff + 1])

            # FFN mm2 + residual + LN2 + store
            w2_r = w2_d[:].rearrange("(c p) n -> p c n", p=P)
            f2 = [pcd.tile([P, D], F32, name=f"f2_{t}", tag=f"f2_{t}")
                  for t in range(NTQ)]
            for dc in range(2):
                dsl = slice(dc * 512, (dc + 1) * 512)
                g2p = [pps.tile([P, 512], F32, name=f"g2p{t}", tag=f"g2p{t}",
                                bufs=1) for t in range(NTQ)]
                for kg in range(FFC // 4):
                    w2t = pwk.tile([P, 4, 512], BF16, name="w2t", tag="w2t",
                                   bufs=3)
                    nc.sync.dma_start(out=w2t,
                                      in_=w2_r[:, 4 * kg:4 * (kg + 1), dsl])
                    for kl in range(4):
                        kc2 = 4 * kg + kl
                        for t in range(NTQ):
                            nc.tensor.matmul(
                                g2p[t], f1[:, kc2, t * P:(t + 1) * P],
                                w2t[:, kl, :], start=(kc2 == 0),
                                stop=(kc2 == FFC - 1))
                for t in range(NTQ):
                    nc.vector.tensor_tensor(f2[t][:, dsl], g2p[t],
                                            h_t[t][:, dsl], ALU.add)
                    nc.vector.tensor_tensor(f2[t][:, dsl], f2[t][:, dsl],
                                            b2_bc[:, dsl], ALU.add)
            for t in range(NTQ):
                layernorm(f2[t], g2_bc, be2_bc, pwk)
                nc.sync.dma_start(out=out_d[t * P:(t + 1) * P, :], in_=f2[t])

    nc.compile()
    return nc


def make_in_maps(cfg, inp):
    """Build per-core input dicts from full (host) inputs."""
    B, L, D, H = cfg["B"], cfg["L"], cfg["D"], cfg["H"]
    NCORES = cfg["NCORES"]
    CPB = NCORES // B
    TOK = L // CPB
    f32 = np.float32
    bf16 = ml_dtypes.bfloat16
    x = np.asarray(inp["x"], f32)
    mask = np.asarray(inp["mask"], bool)
    w = {k: np.asarray(inp[k], f32) for k in
         ("wq", "bq", "wk", "bk", "wv", "bv", "wo", "bo", "w1", "b1",
          "w2", "b2", "ln1_g", "ln1_b", "ln2_g", "ln2_b")}
    shared = dict(wq=w["wq"].astype(bf16), wk=w["wk"].astype(bf16),
                  wv=w["wv"].astype(bf16), wo=w["wo"].astype(bf16),
                  w1=w["w1"].astype(bf16), w2=w["w2"].astype(bf16),
                  bq=w["bq"], bk=w["bk"], bv=w["bv"], bo=w["bo"],
                  b1=w["b1"], b2=w["b2"],
                  g1=w["ln1_g"], be1=w["ln1_b"], g2=w["ln2_g"],
                  be2=w["ln2_b"])
    shared = {k: np.ascontiguousarray(v) for k, v in shared.items()}
    in_maps = []
    for c in range(NCORES):
        b, q0 = c // CPB, (c % CPB) * TOK
        xb = x[b]
        m01 = np.where(mask[b, q0:q0 + TOK, :].T, bf16(0.0), bf16(1.0))
        m = dict(shared)
        m["xt"] = np.ascontiguousarray(xb.T.astype(bf16))
        m["xtq"] = np.ascontiguousarray(xb[q0:q0 + TOK].T.astype(bf16))
        m["xq"] = np.ascontiguousarray(xb[q0:q0 + TOK])
        m["m01"] = np.ascontiguousarray(m01.astype(bf16))
        in_maps.append(m)
    return in_maps


_NC_CACHE = {}
TRACE = False
LAST_RESULTS = None


def _get_nc(key, cfg):
    if key not in _NC_CACHE:
        _NC_CACHE[key] = build_bass(cfg)
    return _NC_CACHE[key]


def kernel(**inputs):
    global LAST_RESULTS
    from concourse.bass_utils import run_bass_kernel_spmd

    cfg = FULL_CFG
    B, L, D = cfg["B"], cfg["L"], cfg["D"]
    NCORES = cfg["NCORES"]
    CPB = NCORES // B
    TOK = L // CPB
    nc = _get_nc("full", cfg)
    in_maps = make_in_maps(cfg, inputs)
    res = run_bass_kernel_spmd(nc, in_maps, core_ids=list(range(NCORES)),
                               trace=TRACE)
    LAST_RESULTS = res
    out = np.empty((B, L, D), np.float32)
    for c in range(NCORES):
        b, q0 = c // CPB, (c % CPB) * TOK
        out[b, q0:q0 + TOK] = res.results[c]["out"]
    return out
